# revision 22
# baseline (speedup 1.0000x reference)
"""Trainium2 Bass kernel for dilated 5x7 conv (128->16ch) + 1x1 (16->16) + 1x1 (16->128).

Strategy (data-parallel, 1 image per core across 8 cores):
  reference: y = conv_dilated(x, w3, dil=(2,3), pad=(4,9)); y = w4@y; y = w5@y
  Host folds w45 = w5 @ w4  [128, 16].

  kh-first decomposition (vs the kw-first v1): stage 1 contracts (c, kh)
  producing (kw,co)=112 channels -- 112/128 PE column utilization and only
  5 matmuls per 8-row chunk (35 total, 15.7k PE-elems vs 22k for kw-first).
  The leftover stage-2 shift is then along the W (free) axis, which makes
  every output chunk depend on exactly ONE stage-1 chunk (no cross-chunk
  row coupling):

  Per core, image x [128, 56, 56] row-padded into SBUF xrp [128, 64, 56]
  (pad rows memset once).  Stage 1 (TensorE), reversed row order, 7 chunks
  of 8 rows: for kh in 0..4 one PSUM-accumulating matmul, lhsT =
  w1p[:, kh, :] [c=128, (kw,co)=112], rhs = xrp[:, a+2kh : b+2kh, :]
  -> P[(kw,co), r, w].  Evacuate PSUM->SBUF p2s (f32->bf16) on ScalarE.
  Stage 2 needs p2a[(kw,co), h, w] = p2[(kw,co), h, w + 3kw - 9] (w-shift
  only).  Rows 8..56: dump evac'd rows into a zero-padded DRAM slab
  [112, rows, 74] (cols 9..65), gather back with a diagonal AP (+3 per kw
  block, legal on the DRAM side); slab zero padding supplies the
  out-of-range zeros, so one uniform rectangular gather per pair and ONE
  K=112 matmul per 8 output rows (lhsT = w2f[(kw,co), o]).
  Rows 0..8 (dependent on the last stage-1 chunk) skip the roundtrip:
  7 col-subrange matmuls (kw=3 opens full-width, others accumulate into
  their valid column window) reading p2s directly.
  Evacuate stage 2 (VectorE, f32->bf16) and DMA out per 8/16 rows.
  PE p-state prewarm: a few dummy matmuls bridge the preamble-to-input-DMA
  gap so the clock ramp starts as early as possible.
  Single-wait discipline: every matmul carries at most one semaphore wait
  (PSUM-reuse or data); extra data deps ride Ldweights or are absorbed by
  tiny preceding PE matmuls reading a sliver of the same writer's region.
"""

import os
import sys

import numpy as np

for _p in ("/opt/trn_rl_repo", "/root/.axon_site/_ro/trn_rl_repo"):
    if os.path.isdir(_p) and _p not in sys.path:
        sys.path.insert(0, _p)

import ml_dtypes  # noqa: E402

import concourse.bass as bass  # noqa: E402
import concourse.tile as tile  # noqa: E402
from concourse.tile_rust import add_dep_helper  # noqa: E402
from concourse import mybir  # noqa: E402
from concourse.bass_utils import run_bass_kernel_spmd  # noqa: E402

N, C, H, W = 8, 128, 56, 56
CO = 16
KH, KW = 5, 7
DH, DW = 2, 3
PH, PW = 4, 9
RP = H + 2 * PH  # 64 padded rows in xrp
WS = W + 2 * PW  # 74 slab width
M1 = KW * CO  # 112 stage-1 output channels (kw, co)
W1C = KH * M1  # 560
# stage-1 chunks in out-row coords, processed in listed (reversed) order
S1_CHUNKS = [(48, 56), (40, 48), (32, 40), (24, 32), (16, 24), (8, 16), (0, 8)]
# x DMAs in x-row coords: rows 44:56 first (chunk 0), then the rest in one
# transfer (lands before chunk 1 needs it); fewer queue entries keeps every
# data-waiting DMA below the 6-deep descriptor ring (ring wait would be a
# 2nd wait and DMA_DIRECT2D fits ONE).
X_DMAS = [(44, 56), (0, 44)]
# dump slabs: (out-row range, stage-1 chunk index whose evac completes it)
DUMPS = [((40, 56), 1), ((24, 40), 3), ((16, 24), 4), ((8, 16), 5)]
# mux col windows: kw -> (src_lo, src_hi, dst_lo, dst_hi)
MUX_KW = []
for _kw in range(KW):
    _slo, _shi = max(0, DW * _kw - PW), min(W, DW * _kw - PW + W)
    _dlo, _dhi = max(0, PW - DW * _kw), min(W, PW - DW * _kw + W)
    MUX_KW.append((_slo, _shi, _dlo, _dhi))
MUX_ORDER = [3, 0, 1, 2, 4, 5, 6]  # kw=3 opens full-width
BF16 = mybir.dt.bfloat16
F32 = mybir.dt.float32

DUM_A = 4  # prewarm dummies bridging preamble -> first input DMA

_NC = None


def _build_nc(attempt=0):
    nc = bass.Bass(
        "TRN2",
        target_bir_lowering=False,
        debug=False,
        enable_asserts=False,
        num_devices=N,
    )
    wx_d = nc.dram_tensor("wx", [C, W1C], BF16, kind="ExternalInput")
    xr_d = nc.dram_tensor("xr", [C, H, W], BF16, kind="ExternalInput")
    wk2_d = nc.dram_tensor("wk2", [M1, (1 + KW) * C], BF16, kind="ExternalInput")
    # flat scratch: len(DUMPS) slabs of [112, 16, 74] + slop for the last
    # gather block's flat-read overrun (<= 3*(KW-1) elements)
    scr_d = nc.dram_tensor(
        "scr", [len(DUMPS) * M1 * 16 * WS + 32], BF16, kind="ExternalInput"
    )
    out_d = nc.dram_tensor("out", [C, H * W], BF16, kind="ExternalOutput")

    with tile.TileContext(nc) as tc:
        for _ in range(attempt):
            nc.sync.nop(nofuse=True)
        with (
            tc.tile_pool(name="const", bufs=1) as constp,
            tc.tile_pool(name="xin", bufs=1) as xinp,
            tc.tile_pool(name="p2s", bufs=1) as p2sp,
            tc.tile_pool(name="p2a", bufs=1) as p2ap,
            tc.tile_pool(name="outs", bufs=1) as outsp,
            tc.tile_pool(name="dum", bufs=1) as dump_,
            tc.tile_pool(name="ps1", bufs=2, space="PSUM") as ps1,
            tc.tile_pool(name="ps2", bufs=4, space="PSUM") as ps2g,
            tc.tile_pool(name="psm", bufs=1, space="PSUM") as psm,
            tc.tile_pool(name="psh", bufs=1, space="PSUM") as psh,
        ):
            in_dmas = []
            aux_dmas = []
            out_dmas = []
            # dummy source for PE prewarm; memset LAST on gpsimd so the first
            # dummy's dep imports the whole gpsimd clock (incl. pad memsets).
            dum_t = dump_.tile([C, 448], BF16, tag="dum")

            xrp_t = xinp.tile([C, RP, W], BF16, tag="xrp")
            # xrp pad rows (outside [PH, PH+H)) are known zero
            nc.gpsimd.memset(xrp_t[:, 0:PH, :], 0)
            nc.gpsimd.memset(xrp_t[:, PH + H : RP, :], 0)
            last_pool = nc.gpsimd.memset(dum_t[:], 0)

            wx_t = constp.tile([C, W1C], BF16, tag="wx")
            w1_t = wx_t[:].rearrange("c (kh m) -> c kh m", kh=KH)
            wk2_t = constp.tile([M1, (1 + KW) * C], BF16, tag="wk2")
            w2f_t = wk2_t[:, 0:C]  # [112, 128]
            # w45z[kw]: [112, 128], rows 16kw..16kw+16 = w45.T, else 0
            w45z_t = [wk2_t[:, (1 + kw) * C : (2 + kw) * C] for kw in range(KW)]

            # input DMAs on the scalar (Act) HW queue: Act is idle until the
            # first evac (~3us after these kicks), and this keeps the sync
            # queue free for the 4 gathers + 2 output DMAs (6 = ring depth).
            x_dmas = []
            for a, b in X_DMAS:
                d = nc.scalar.dma_start(
                    xrp_t[:, PH + a : PH + b, :], xr_d.ap()[:, a:b, :]
                )
                x_dmas.append(d)
                in_dmas.append(d)
                if a == 44:  # weights ride right behind the first x rows
                    wx_dma = nc.scalar.dma_start(wx_t[:], wx_d.ap())
                    in_dmas.append(wx_dma)
            # stage-2 weights via SWDGE; tiny, needed ~6us in
            wk2_dma = nc.gpsimd.dma_start(wk2_t[:], wk2_d.ap())
            in_dmas.append(wk2_dma)

            p2s_t = p2sp.tile([M1, H, W], BF16)
            # gathered+shifted stage-1 rows, still 74 wide: the gather is a
            # flat contiguous read per (kw,co) block offset by +3kw, so row
            # tails hold wrap garbage -- matmuls only read cols 0..56.
            p2a_t = p2ap.tile([M1, H - 8, WS], BF16)  # out rows 8..56
            outsb_t = outsp.tile([C, H * W], BF16)
            out_ap = out_d.ap()
            scr_ap = scr_d.ap()

            # dummy/absorb PSUM tile; writers ordered by PE program order
            dpt = psh.tile([C, 6, W], F32, tag="dum")

            def dummy_mm(rhs):
                return nc.tensor.matmul(
                    dpt[0:1, 0:1, 0:8].squeeze(1),
                    rhs[:, 0:1],
                    rhs[:, 0:8],
                    start=True,
                    stop=True,
                    skip_group_check=True,
                )

            # PE prewarm from the end of the NEFF preamble until input lands
            for _ in range(DUM_A):
                nc.tensor.matmul(
                    dpt[:, 0:6, :],
                    dum_t[:, 0:128],
                    dum_t[:, 0:336],
                    start=True,
                    stop=True,
                    skip_group_check=True,
                )

            def gather(di):
                (a, b), _ = DUMPS[di]
                n = b - a
                src = scr_ap.copy()
                v = src.ap
                v.clear()
                v.extend(
                    [
                        [CO * 16 * WS + DW, KW],
                        [16 * WS, CO],
                        [1, n * WS],
                    ]
                )
                src.offset = di * M1 * 16 * WS
                d = nc.sync.dma_start(p2a_t[:, a - 8 : b - 8, :], src)
                aux_dmas.append(d)
                return d

            def dump_ap(di, n):
                dst = scr_ap.copy()
                v = dst.ap
                v.clear()
                v.extend([[16 * WS, M1], [WS, n], [1, W]])
                dst.offset = di * M1 * 16 * WS + PW
                return dst

            # ---- stage 1 (reversed), with dumps+gathers trailing evacs ----
            evacs = []
            for ci, (a, b) in enumerate(S1_CHUNKS):
                pt = ps1.tile([M1, 8, W], F32, tag="p1")
                # kh order: opener must read only already-imported regions so
                # it carries just the PSUM-reuse wait (Matmult fits ONE wait).
                # Chunks 3 and 5 start right below an x-DMA boundary (padded
                # rows 32 / 16), so their kh=2 window would first-touch the
                # new region -- open with kh=4 (their topmost, oldest rows)
                # and let kh=0 carry the new DMA's wait on its free slot.
                if ci == 0:
                    kh_order = [0, 1, 2, 3, 4]
                elif ci in (3, 5):
                    kh_order = [4, 0, 1, 2, 3]
                else:
                    kh_order = [2, 0, 1, 3, 4]
                for i, kh in enumerate(kh_order):
                    nc.tensor.matmul(
                        pt[:],
                        w1_t[:, kh, :],
                        xrp_t[:, a + DH * kh : b + DH * kh, :],
                        start=(i == 0),
                        stop=(i == KH - 1),
                        skip_group_check=True,
                    )
                ev = nc.scalar.copy(p2s_t[:, a:b, :], pt[:])
                evacs.append(ev)
                for di, ((dlo, dhi), after) in enumerate(DUMPS):
                    if after == ci:
                        aux_dmas.append(
                            nc.gpsimd.dma_start(
                                dump_ap(di, dhi - dlo),
                                p2s_t[:, dlo:dhi, :],
                            )
                        )
                        gather(di)

            # ---- stage 2 ----
            s2_mms = []
            s2_cps = []
            last_cp = None

            def emit_pair(di, absorb_gather):
                nonlocal last_cp
                (a, b), _ = DUMPS[di]
                if absorb_gather:
                    ab = dummy_mm(p2a_t[:, a - 8 : a - 7, :].squeeze(1))
                    s2_mms.append(ab)
                else:
                    ab = None
                for j in range(a, b, 8):
                    qt = ps2g.tile([C, 8, W], F32, tag="p2")
                    mm = nc.tensor.matmul(
                        qt[:],
                        w2f_t,
                        p2a_t[:, j - 8 : j, 0:W],
                        start=True,
                        stop=True,
                    )
                    s2_mms.append(mm)
                    if ab is not None and j == a:
                        add_dep_helper(mm.ins, ab.ins, sync=False, reason="order")
                    last_cp = nc.vector.tensor_copy(
                        outsb_t[:, j * W : (j + 8) * W], qt[:]
                    )
                    s2_cps.append(last_cp)

            def emit_out_dma(a, b, eng=None):
                out_dmas.append(
                    (eng or nc.gpsimd).dma_start(
                        out_ap[:, a * W : b * W], outsb_t[:, a * W : b * W]
                    )
                )

            # all gathers landed during stage 1; pairs 0/1 use fresh PSUM
            # bufs (gather dep rides the matmul's single wait), pairs 2/3
            # also carry a PSUM-reuse wait so their gather dep is absorbed.
            emit_pair(0, absorb_gather=False)
            emit_pair(1, absorb_gather=False)
            emit_pair(2, absorb_gather=True)
            emit_pair(3, absorb_gather=True)
            # outputs ride the gpsimd SWDGE queue (no descriptor-ring waits
            # observed there); sync carries only the 4 gathers.
            emit_out_dma(8, 56)

            # tail mux: rows 0..8 straight from p2s with col-subrange matmuls
            qt = psm.tile([C, 8, W], F32, tag="pm")
            last_mm = None
            for i, kw in enumerate(MUX_ORDER):
                slo, shi, dlo, dhi = MUX_KW[kw]
                last_mm = nc.tensor.matmul(
                    qt[:, :, dlo:dhi],
                    w45z_t[kw],
                    p2s_t[:, 0:8, slo:shi],
                    start=(i == 0),
                    stop=(i == len(MUX_ORDER) - 1),
                    skip_group_check=True,
                )
                s2_mms.append(last_mm)
            last_cp = nc.vector.tensor_copy(outsb_t[:, 0 : 8 * W], qt[:])
            s2_cps.append(last_cp)
            emit_out_dma(0, 8)

            # final PE dummy reads the last output copy's region: it waits the
            # DVE copy and is nosync-pinned after every stage-2 op, so PE's
            # final tick transitively implies all compute.  The SP absorb nops
            # then cover it plus the DMAs, leaving the tail Drain <= 1 wait.
            chain = dummy_mm(outsb_t[:, 0:448])
            for m in s2_mms + s2_cps:
                add_dep_helper(chain.ins, m.ins, sync=False, reason="tail")
            for dep in (
                [chain]
                + in_dmas
                + aux_dmas
                + out_dmas
                + [evacs[-1], last_pool]
            ):
                nop = nc.sync.nop(nofuse=True)
                add_dep_helper(nop.ins, dep.ins, sync=True, reason="absorb tick")
    return nc


def _get_nc():
    global _NC
    if _NC is None:
        _NC = _build_nc()
    return _NC


def _prep_inputs(x, w3, w4, w5):
    w45 = (w5.astype(np.float64) @ w4.astype(np.float64)).astype(np.float32)
    # w1p[c, kh, kw*CO+co] = w3[co, c, kh, kw]
    w1p = np.transpose(w3, (1, 2, 3, 0)).reshape(C, W1C)
    # w2f[kw*CO+co, o] = w45[o, co]
    w2f = np.tile(w45.T, (KW, 1))
    wk2 = np.zeros((M1, (1 + KW) * C), np.float32)
    wk2[:, 0:C] = w2f
    for kw in range(KW):
        wk2[CO * kw : CO * (kw + 1), (1 + kw) * C : (2 + kw) * C] = w45.T
    wx = w1p.astype(ml_dtypes.bfloat16)
    wk2 = wk2.astype(ml_dtypes.bfloat16)
    xr = x.astype(ml_dtypes.bfloat16)
    return wx, xr, wk2


def kernel(x, w3, w4, w5, trace=False):
    x = np.asarray(x, np.float32)
    w3 = np.asarray(w3, np.float32)
    w4 = np.asarray(w4, np.float32)
    w5 = np.asarray(w5, np.float32)
    wx, xr, wk2 = _prep_inputs(x, w3, w4, w5)
    scr0 = np.zeros(len(DUMPS) * M1 * 16 * WS + 32, ml_dtypes.bfloat16)
    in_maps = [
        {
            "wx": wx,
            "xr": np.ascontiguousarray(xr[n]),
            "wk2": wk2,
            "scr": scr0,
        }
        for n in range(N)
    ]
    global _NC
    res = None
    last_err = None
    for attempt in range(6):
        if _NC is None:
            _NC = _build_nc(attempt)
        try:
            res = run_bass_kernel_spmd(
                _NC, in_maps, core_ids=list(range(N)), trace=trace
            )
            break
        except Exception as e:  # compile-schedule flake: rebuild perturbed
            last_err = e
            _NC = None
    if res is None:
        raise last_err
    out = np.stack(
        [
            np.asarray(res.results[n]["out"])
            .astype(np.float32)
            .reshape(C, H, W)
            for n in range(N)
        ]
    )
    if trace:
        return out, res
    return out


# revision 34
# speedup vs baseline: 1.4143x; 1.4143x over previous
"""Trainium2 Bass kernel for dilated 5x7 conv (128->16ch) + 1x1 (16->16) + 1x1 (16->128).

Strategy (data-parallel, 1 image per core across 8 cores):
  reference: y = conv_dilated(x, w3, dil=(2,3), pad=(4,9)); y = w4@y; y = w5@y
  Host folds w45 = w5 @ w4  [128, 16].

  kh-first decomposition (vs the kw-first v1): stage 1 contracts (c, kh)
  producing (kw,co)=112 channels -- 112/128 PE column utilization and only
  5 matmuls per 8-row chunk (35 total, 15.7k PE-elems vs 22k for kw-first).
  The leftover stage-2 shift is then along the W (free) axis, which makes
  every output chunk depend on exactly ONE stage-1 chunk (no cross-chunk
  row coupling):

  Per core, image x [128, 56, 56] row-padded into SBUF xrp [128, 64, 56]
  (pad rows memset once).  Stage 1 (TensorE), reversed row order, 7 chunks
  of 8 rows: for kh in 0..4 one PSUM-accumulating matmul, lhsT =
  w1p[:, kh, :] [c=128, (kw,co)=112], rhs = xrp[:, a+2kh : b+2kh, :]
  -> P[(kw,co), r, w].  Evacuate PSUM->SBUF p2s (f32->bf16) on ScalarE.
  Stage 2 needs p2a[(kw,co), h, w] = p2[(kw,co), h, w + 3kw - 9] (w-shift
  only).  Rows 8..56: dump evac'd rows into a zero-padded DRAM slab
  [112, rows, 74] (cols 9..65), gather back with a diagonal AP (+3 per kw
  block, legal on the DRAM side); slab zero padding supplies the
  out-of-range zeros, so one uniform rectangular gather per pair and ONE
  K=112 matmul per 8 output rows (lhsT = w2f[(kw,co), o]).
  Rows 0..8 (dependent on the last stage-1 chunk) skip the roundtrip:
  7 col-subrange matmuls (kw=3 opens full-width, others accumulate into
  their valid column window) reading p2s directly.
  Evacuate stage 2 (VectorE, f32->bf16) and DMA out per 8/16 rows.
  PE p-state prewarm: a few dummy matmuls bridge the preamble-to-input-DMA
  gap so the clock ramp starts as early as possible.
  Single-wait discipline: every matmul carries at most one semaphore wait
  (PSUM-reuse or data); extra data deps ride Ldweights or are absorbed by
  tiny preceding PE matmuls reading a sliver of the same writer's region.
"""

import os
import sys

import numpy as np

for _p in ("/opt/trn_rl_repo", "/root/.axon_site/_ro/trn_rl_repo"):
    if os.path.isdir(_p) and _p not in sys.path:
        sys.path.insert(0, _p)

import ml_dtypes  # noqa: E402

import concourse.bass as bass  # noqa: E402
import concourse.tile as tile  # noqa: E402
from concourse.tile_rust import add_dep_helper  # noqa: E402
from concourse import mybir  # noqa: E402
from concourse.bass_utils import run_bass_kernel_spmd  # noqa: E402

N, C, H, W = 8, 128, 56, 56
CO = 16
KH, KW = 5, 7
DH, DW = 2, 3
PH, PW = 4, 9
RP = H + 2 * PH  # 64 padded rows in xrp
WS = W + 2 * PW  # 74 slab width
M1 = KW * CO  # 112 stage-1 output channels (kw, co)
W1C = KH * M1  # 560
# stage-1 chunks in out-row coords, processed in listed (reversed) order
S1_CHUNKS = [(48, 56), (40, 48), (32, 40), (24, 32), (16, 24), (8, 16), (0, 8)]
# x DMAs in x-row coords: rows 44:56 first (chunk 0), then the rest in one
# transfer (lands before chunk 1 needs it); fewer queue entries keeps every
# data-waiting DMA below the 6-deep descriptor ring (ring wait would be a
# 2nd wait and DMA_DIRECT2D fits ONE).
X_DMAS = [(44, 56), (0, 44)]
# dump slabs: (out-row range, stage-1 chunk index whose evac completes it)
DUMPS = [((40, 56), 1), ((24, 40), 3), ((16, 24), 4), ((8, 16), 5)]
# mux col windows: kw -> (src_lo, src_hi, dst_lo, dst_hi)
MUX_KW = []
for _kw in range(KW):
    _slo, _shi = max(0, DW * _kw - PW), min(W, DW * _kw - PW + W)
    _dlo, _dhi = max(0, PW - DW * _kw), min(W, PW - DW * _kw + W)
    MUX_KW.append((_slo, _shi, _dlo, _dhi))
MUX_ORDER = [3, 0, 1, 2, 4, 5, 6]  # kw=3 opens full-width
BF16 = mybir.dt.bfloat16
F32 = mybir.dt.float32

DUM_A = 4  # prewarm dummies bridging preamble -> first input DMA

_NC = None


def _build_nc(attempt=0):
    nc = bass.Bass(
        "TRN2",
        target_bir_lowering=False,
        debug=False,
        enable_asserts=False,
        num_devices=N,
    )
    wx_d = nc.dram_tensor("wx", [C, W1C], BF16, kind="ExternalInput")
    xr_d = nc.dram_tensor("xr", [C, H, W], BF16, kind="ExternalInput")
    wk2_d = nc.dram_tensor("wk2", [M1, (1 + KW) * C], BF16, kind="ExternalInput")
    # flat scratch: len(DUMPS) slabs of [112, 16, 74] + slop for the last
    # gather block's flat-read overrun (<= 3*(KW-1) elements)
    scr_d = nc.dram_tensor(
        "scr", [len(DUMPS) * M1 * 16 * WS + 32], BF16, kind="ExternalInput"
    )
    out_d = nc.dram_tensor("out", [C, H * W], BF16, kind="ExternalOutput")

    with tile.TileContext(nc) as tc:
        for _ in range(attempt):
            nc.sync.nop(nofuse=True)
        with (
            tc.tile_pool(name="const", bufs=1) as constp,
            tc.tile_pool(name="xin", bufs=1) as xinp,
            tc.tile_pool(name="p2s", bufs=1) as p2sp,
            tc.tile_pool(name="p2a", bufs=1) as p2ap,
            tc.tile_pool(name="outs", bufs=1) as outsp,
            tc.tile_pool(name="dum", bufs=1) as dump_,
            tc.tile_pool(name="ps1", bufs=2, space="PSUM") as ps1,
            tc.tile_pool(name="ps2", bufs=4, space="PSUM") as ps2g,
            tc.tile_pool(name="psm", bufs=1, space="PSUM") as psm,
            tc.tile_pool(name="psh", bufs=1, space="PSUM") as psh,
        ):
            in_dmas = []
            aux_dmas = []
            out_dmas = []
            # dummy source for PE prewarm; memset LAST on gpsimd so the first
            # dummy's dep imports the whole gpsimd clock (incl. pad memsets).
            dum_t = dump_.tile([C, 448], BF16, tag="dum")

            xrp_t = xinp.tile([C, RP, W], BF16, tag="xrp")
            # stage-1 output, 74 wide with zero pad cols: evacs write cols
            # 9..65, so dumps are fully contiguous per partition (fast DMA)
            # and the mux reads shifted windows [3kw, 3kw+56) directly.
            p2s_t = p2sp.tile([M1, H, WS], BF16)
            # xrp pad rows (outside [PH, PH+H)) and p2s pad cols are known
            # zero; dum_t is memset LAST so the first dummy matmul's dep
            # imports the whole gpsimd clock (pad memsets ride along).
            nc.gpsimd.memset(xrp_t[:, 0:PH, :], 0)
            nc.gpsimd.memset(xrp_t[:, PH + H : RP, :], 0)
            nc.gpsimd.memset(p2s_t[:, :, 0:PW], 0)
            pad_ms = nc.gpsimd.memset(p2s_t[:, :, PW + W : WS], 0)
            last_pool = nc.gpsimd.memset(dum_t[:], 0)

            wx_t = constp.tile([C, W1C], BF16, tag="wx")
            w1_t = wx_t[:].rearrange("c (kh m) -> c kh m", kh=KH)
            wk2_t = constp.tile([M1, (1 + KW) * C], BF16, tag="wk2")
            w2f_t = wk2_t[:, 0:C]  # [112, 128]
            # w45z[kw]: [112, 128], rows 16kw..16kw+16 = w45.T, else 0
            w45z_t = [wk2_t[:, (1 + kw) * C : (2 + kw) * C] for kw in range(KW)]

            # input DMAs on the scalar (Act) HW queue: Act is idle until the
            # first evac (~3us after these kicks), and this keeps the sync
            # queue free for the 4 gathers + 2 output DMAs (6 = ring depth).
            # weights first: the queue round-robins packets of all queued
            # transfers across its 16 engines, so whatever is queued first
            # completes first -- and w1/x44 gate the first real matmul.
            wx_dma = nc.scalar.dma_start(wx_t[:], wx_d.ap())
            in_dmas.append(wx_dma)
            x_dmas = []
            for a, b in X_DMAS:
                d = nc.scalar.dma_start(
                    xrp_t[:, PH + a : PH + b, :], xr_d.ap()[:, a:b, :]
                )
                x_dmas.append(d)
                in_dmas.append(d)

            # stage-2 weights via SWDGE; tiny, needed ~6us in.  Its free wait
            # slot absorbs the p2s pad-memset dep so the dumps (which read
            # the pads) carry only their Act evac wait.
            wk2_dma = nc.gpsimd.dma_start(wk2_t[:], wk2_d.ap())
            in_dmas.append(wk2_dma)
            add_dep_helper(wk2_dma.ins, pad_ms.ins, sync=True, reason="import")

            # gathered+shifted stage-1 rows, still 74 wide: the gather is a
            # flat contiguous read per (kw,co) block offset by +3kw, so row
            # tails hold wrap garbage -- matmuls only read cols 0..56.
            p2a_t = p2ap.tile([M1, H - 8, WS], BF16)  # out rows 8..56
            outsb_t = outsp.tile([C, H * W], BF16)
            out_ap = out_d.ap()
            scr_ap = scr_d.ap()

            # dummy/absorb PSUM tile; writers ordered by PE program order
            dpt = psh.tile([C, 6, W], F32, tag="dum")

            def dummy_mm(rhs):
                return nc.tensor.matmul(
                    dpt[0:1, 0:1, 0:8].squeeze(1),
                    rhs[:, 0:1],
                    rhs[:, 0:8],
                    start=True,
                    stop=True,
                    skip_group_check=True,
                )

            # PE prewarm from the end of the NEFF preamble until input lands
            for _ in range(DUM_A):
                nc.tensor.matmul(
                    dpt[:, 0:6, :],
                    dum_t[:, 0:128],
                    dum_t[:, 0:336],
                    start=True,
                    stop=True,
                    skip_group_check=True,
                )

            def gather(di):
                (a, b), _ = DUMPS[di]
                n = b - a
                src = scr_ap.copy()
                v = src.ap
                v.clear()
                v.extend(
                    [
                        [CO * 16 * WS + DW, KW],
                        [16 * WS, CO],
                        [1, n * WS],
                    ]
                )
                src.offset = di * M1 * 16 * WS
                d = nc.sync.dma_start(p2a_t[:, a - 8 : b - 8, :], src)
                aux_dmas.append(d)
                return d

            def dump_ap(di, n):
                dst = scr_ap.copy()
                v = dst.ap
                v.clear()
                v.extend([[16 * WS, M1], [1, n * WS]])
                dst.offset = di * M1 * 16 * WS
                return dst

            # ---- stage 1 (reversed), with dumps+gathers trailing evacs ----
            evacs = []
            for ci, (a, b) in enumerate(S1_CHUNKS):
                pt = ps1.tile([M1, 8, W], F32, tag="p1")
                # kh order: opener must read only already-imported regions so
                # it carries just the PSUM-reuse wait (Matmult fits ONE wait).
                # Chunks 3 and 5 start right below an x-DMA boundary (padded
                # rows 32 / 16), so their kh=2 window would first-touch the
                # new region -- open with kh=4 (their topmost, oldest rows)
                # and let kh=0 carry the new DMA's wait on its free slot.
                if ci == 0:
                    kh_order = [0, 1, 2, 3, 4]
                elif ci in (3, 5):
                    kh_order = [4, 0, 1, 2, 3]
                else:
                    kh_order = [2, 0, 1, 3, 4]
                for i, kh in enumerate(kh_order):
                    nc.tensor.matmul(
                        pt[:],
                        w1_t[:, kh, :],
                        xrp_t[:, a + DH * kh : b + DH * kh, :],
                        start=(i == 0),
                        stop=(i == KH - 1),
                        skip_group_check=True,
                    )
                ev = nc.scalar.copy(p2s_t[:, a:b, PW : PW + W], pt[:])
                evacs.append(ev)
                for di, ((dlo, dhi), after) in enumerate(DUMPS):
                    if after == ci:
                        aux_dmas.append(
                            nc.gpsimd.dma_start(
                                dump_ap(di, dhi - dlo),
                                p2s_t[:, dlo:dhi, :],
                            )
                        )
                        gather(di)

            # ---- stage 2 ----
            s2_mms = []
            s2_cps = []
            last_cp = None

            def emit_pair(di, absorb_gather):
                nonlocal last_cp
                (a, b), _ = DUMPS[di]
                if absorb_gather:
                    ab = dummy_mm(p2a_t[:, a - 8 : a - 7, :].squeeze(1))
                    s2_mms.append(ab)
                else:
                    ab = None
                for j in range(a, b, 8):
                    qt = ps2g.tile([C, 8, W], F32, tag="p2")
                    mm = nc.tensor.matmul(
                        qt[:],
                        w2f_t,
                        p2a_t[:, j - 8 : j, 0:W],
                        start=True,
                        stop=True,
                    )
                    s2_mms.append(mm)
                    if ab is not None and j == a:
                        add_dep_helper(mm.ins, ab.ins, sync=False, reason="order")
                    last_cp = nc.vector.tensor_copy(
                        outsb_t[:, j * W : (j + 8) * W], qt[:]
                    )
                    s2_cps.append(last_cp)

            def emit_out_dma(a, b, eng=None):
                out_dmas.append(
                    (eng or nc.gpsimd).dma_start(
                        out_ap[:, a * W : b * W], outsb_t[:, a * W : b * W]
                    )
                )

            # all gathers landed during stage 1; pairs 0/1 use fresh PSUM
            # bufs (gather dep rides the matmul's single wait), pairs 2/3
            # also carry a PSUM-reuse wait so their gather dep is absorbed.
            emit_pair(0, absorb_gather=False)
            emit_pair(1, absorb_gather=False)
            emit_pair(2, absorb_gather=True)
            emit_pair(3, absorb_gather=True)
            # outputs ride the gpsimd SWDGE queue (no descriptor-ring waits
            # observed there); sync carries only the 4 gathers.
            emit_out_dma(8, 56)

            # tail mux: rows 0..8 straight from p2s -- the 74-wide zero pad
            # makes every kw's shifted window [3kw, 3kw+56) valid, so all 7
            # accumulating matmuls are uniform full-width.
            qt = psm.tile([C, 8, W], F32, tag="pm")
            last_mm = None
            for i, kw in enumerate(MUX_ORDER):
                last_mm = nc.tensor.matmul(
                    qt[:],
                    w45z_t[kw],
                    p2s_t[:, 0:8, DW * kw : DW * kw + W],
                    start=(i == 0),
                    stop=(i == len(MUX_ORDER) - 1),
                    skip_group_check=True,
                )
                s2_mms.append(last_mm)
            last_cp = nc.vector.tensor_copy(outsb_t[:, 0 : 8 * W], qt[:])
            s2_cps.append(last_cp)
            emit_out_dma(0, 8)

            # final PE dummy reads the last output copy's region: it waits the
            # DVE copy and is nosync-pinned after every stage-2 op, so PE's
            # final tick transitively implies all compute.  The SP absorb nops
            # then cover it plus the DMAs, leaving the tail Drain <= 1 wait.
            chain = dummy_mm(outsb_t[:, 0:448])
            for m in s2_mms + s2_cps:
                add_dep_helper(chain.ins, m.ins, sync=False, reason="tail")
            for dep in (
                [chain]
                + in_dmas
                + aux_dmas
                + out_dmas
                + [evacs[-1], last_pool]
            ):
                nop = nc.sync.nop(nofuse=True)
                add_dep_helper(nop.ins, dep.ins, sync=True, reason="absorb tick")
    return nc


def _get_nc():
    global _NC
    if _NC is None:
        _NC = _build_nc()
    return _NC


def _prep_inputs(x, w3, w4, w5):
    w45 = (w5.astype(np.float64) @ w4.astype(np.float64)).astype(np.float32)
    # w1p[c, kh, kw*CO+co] = w3[co, c, kh, kw]
    w1p = np.transpose(w3, (1, 2, 3, 0)).reshape(C, W1C)
    # w2f[kw*CO+co, o] = w45[o, co]
    w2f = np.tile(w45.T, (KW, 1))
    wk2 = np.zeros((M1, (1 + KW) * C), np.float32)
    wk2[:, 0:C] = w2f
    for kw in range(KW):
        wk2[CO * kw : CO * (kw + 1), (1 + kw) * C : (2 + kw) * C] = w45.T
    wx = w1p.astype(ml_dtypes.bfloat16)
    wk2 = wk2.astype(ml_dtypes.bfloat16)
    xr = x.astype(ml_dtypes.bfloat16)
    return wx, xr, wk2


def kernel(x, w3, w4, w5, trace=False):
    x = np.asarray(x, np.float32)
    w3 = np.asarray(w3, np.float32)
    w4 = np.asarray(w4, np.float32)
    w5 = np.asarray(w5, np.float32)
    wx, xr, wk2 = _prep_inputs(x, w3, w4, w5)
    scr0 = np.zeros(len(DUMPS) * M1 * 16 * WS + 32, ml_dtypes.bfloat16)
    in_maps = [
        {
            "wx": wx,
            "xr": np.ascontiguousarray(xr[n]),
            "wk2": wk2,
            "scr": scr0,
        }
        for n in range(N)
    ]
    global _NC
    res = None
    last_err = None
    for attempt in range(6):
        if _NC is None:
            _NC = _build_nc(attempt)
        try:
            res = run_bass_kernel_spmd(
                _NC, in_maps, core_ids=list(range(N)), trace=trace
            )
            break
        except Exception as e:  # compile-schedule flake: rebuild perturbed
            last_err = e
            _NC = None
    if res is None:
        raise last_err
    out = np.stack(
        [
            np.asarray(res.results[n]["out"])
            .astype(np.float32)
            .reshape(C, H, W)
            for n in range(N)
        ]
    )
    if trace:
        return out, res
    return out


# revision 41
# speedup vs baseline: 1.8249x; 1.2903x over previous
"""Trainium2 Bass kernel for dilated 5x7 conv (128->16ch) + 1x1 (16->16) + 1x1 (16->128).

Strategy (data-parallel, 1 image per core across 8 cores):
  reference: y = conv_dilated(x, w3, dil=(2,3), pad=(4,9)); y = w4@y; y = w5@y
  Host folds w45 = w5 @ w4  [128, 16].

  kh-first decomposition (vs the kw-first v1): stage 1 contracts (c, kh)
  producing (kw,co)=112 channels -- 112/128 PE column utilization and only
  5 matmuls per 8-row chunk (35 total, 15.7k PE-elems vs 22k for kw-first).
  The leftover stage-2 shift is then along the W (free) axis, which makes
  every output chunk depend on exactly ONE stage-1 chunk (no cross-chunk
  row coupling):

  Per core, image x [128, 56, 56] row-padded into SBUF xrp [128, 64, 56]
  (pad rows memset once).  Stage 1 (TensorE), reversed row order, 7 chunks
  of 8 rows: for kh in 0..4 one PSUM-accumulating matmul, lhsT =
  w1p[:, kh, :] [c=128, (kw,co)=112], rhs = xrp[:, a+2kh : b+2kh, :]
  -> P[(kw,co), r, w].  Evacuate PSUM->SBUF p2s (f32->bf16) on ScalarE.
  Stage 2 needs p2a[(kw,co), h, w] = p2[(kw,co), h, w + 3kw - 9] (w-shift
  only).  Rows 8..56: dump evac'd rows into a zero-padded DRAM slab
  [112, rows, 74] (cols 9..65), gather back with a diagonal AP (+3 per kw
  block, legal on the DRAM side); slab zero padding supplies the
  out-of-range zeros, so one uniform rectangular gather per pair and ONE
  K=112 matmul per 8 output rows (lhsT = w2f[(kw,co), o]).
  Rows 0..8 (dependent on the last stage-1 chunk) skip the roundtrip:
  7 col-subrange matmuls (kw=3 opens full-width, others accumulate into
  their valid column window) reading p2s directly.
  Evacuate stage 2 (VectorE, f32->bf16) and DMA out per 8/16 rows.
  PE p-state prewarm: a few dummy matmuls bridge the preamble-to-input-DMA
  gap so the clock ramp starts as early as possible.
  Single-wait discipline: every matmul carries at most one semaphore wait
  (PSUM-reuse or data); extra data deps ride Ldweights or are absorbed by
  tiny preceding PE matmuls reading a sliver of the same writer's region.
"""

import os
import sys

import numpy as np

for _p in ("/opt/trn_rl_repo", "/root/.axon_site/_ro/trn_rl_repo"):
    if os.path.isdir(_p) and _p not in sys.path:
        sys.path.insert(0, _p)

import ml_dtypes  # noqa: E402

import concourse.bass as bass  # noqa: E402
import concourse.tile as tile  # noqa: E402
from concourse.tile_rust import add_dep_helper  # noqa: E402
from concourse import mybir  # noqa: E402
from concourse.bass_utils import run_bass_kernel_spmd  # noqa: E402

N, C, H, W = 8, 128, 56, 56
CO = 16
KH, KW = 5, 7
DH, DW = 2, 3
PH, PW = 4, 9
RP = H + 2 * PH  # 64 padded rows in xrp
WS = W + 2 * PW  # 74 slab width
M1 = KW * CO  # 112 stage-1 output channels (kw, co)
W1C = KH * M1  # 560
SLAB = M1 * 16 * WS + 32  # scratch slab stride (incl. gather-overrun slop)
# stage-1 chunks in out-row coords, processed in listed (reversed) order
S1_CHUNKS = [(48, 56), (40, 48), (32, 40), (24, 32), (16, 24), (8, 16), (0, 8)]
# x DMAs in x-row coords, first-needed first (split so chunk 1 never stalls
# on the input stream and the PE p-state ramp is not paused by idle gaps)
X_DMAS = [(44, 56), (28, 44), (0, 28)]
# dump slabs: (out-row range, stage-1 chunk index whose evac completes it).
# Both are 16 rows: the gather's co-dim stride then equals its flat run
# (16*74), so the AP merges into 7 descriptors of 16*16*74 elements each.
DUMPS = [((40, 56), 1), ((24, 40), 3)]
# rows 0..24 skip the DRAM roundtrip: per 8-row group, 7 accumulating
# matmuls read shifted windows [3kw, 3kw+56) straight from the 74-wide p2s
# (zero pad cols supply the out-of-range zeros).  Listed in emission order
# with the stage-1 chunk index whose evac gates them.
MUX_ROWS = [((16, 24), 4), ((8, 16), 5), ((0, 8), 6)]
BF16 = mybir.dt.bfloat16
F32 = mybir.dt.float32

DUM_A = 4  # prewarm dummies bridging preamble -> first input DMA

_NC = None


def _build_nc(attempt=0):
    nc = bass.Bass(
        "TRN2",
        target_bir_lowering=False,
        debug=False,
        enable_asserts=False,
        num_devices=N,
    )
    wx_d = nc.dram_tensor("wx", [C, W1C], BF16, kind="ExternalInput")
    xr_d = nc.dram_tensor("xr", [C, H, W], BF16, kind="ExternalInput")
    wk2_d = nc.dram_tensor("wk2", [M1, (1 + KW) * C], BF16, kind="ExternalInput")
    # flat scratch: len(DUMPS) slabs of [112, 16, 74], each with its own
    # slop for the gather's flat-read overrun (<= 3*(KW-1) elements) so the
    # overrun never aliases the next slab (would add a WAR wait to dumps)
    scr_d = nc.dram_tensor(
        "scr", [len(DUMPS) * SLAB], BF16, kind="ExternalInput"
    )
    out_d = nc.dram_tensor("out", [C, H * W], BF16, kind="ExternalOutput")

    with tile.TileContext(nc) as tc:
        for _ in range(attempt):
            nc.sync.nop(nofuse=True)
        with (
            tc.tile_pool(name="const", bufs=1) as constp,
            tc.tile_pool(name="xin", bufs=1) as xinp,
            tc.tile_pool(name="p2s", bufs=1) as p2sp,
            tc.tile_pool(name="p2a", bufs=1) as p2ap,
            tc.tile_pool(name="outs", bufs=1) as outsp,
            tc.tile_pool(name="dum", bufs=1) as dump_,
            tc.tile_pool(name="ps1", bufs=2, space="PSUM") as ps1,
            tc.tile_pool(name="ps2", bufs=2, space="PSUM") as ps2g,
            tc.tile_pool(name="psm", bufs=3, space="PSUM") as psm,
            tc.tile_pool(name="psh", bufs=1, space="PSUM") as psh,
        ):
            in_dmas = []
            aux_dmas = []
            out_dmas = []
            # dummy source for PE prewarm; memset LAST on gpsimd so the first
            # dummy's dep imports the whole gpsimd clock (incl. pad memsets).
            dum_t = dump_.tile([C, 448], BF16, tag="dum")

            xrp_t = xinp.tile([C, RP, W], BF16, tag="xrp")
            # stage-1 output, 74 wide with zero pad cols: evacs write cols
            # 9..65, so dumps are fully contiguous per partition (fast DMA)
            # and the mux reads shifted windows [3kw, 3kw+56) directly.
            p2s_t = p2sp.tile([M1, H, WS], BF16)
            # xrp pad rows (outside [PH, PH+H)) and p2s pad cols are known
            # zero; dum_t is memset LAST so the first dummy matmul's dep
            # imports the whole gpsimd clock (pad memsets ride along).
            nc.gpsimd.memset(xrp_t[:, 0:PH, :], 0)
            nc.gpsimd.memset(xrp_t[:, PH + H : RP, :], 0)
            nc.gpsimd.memset(p2s_t[:, :, 0:PW], 0)
            pad_ms = nc.gpsimd.memset(p2s_t[:, :, PW + W : WS], 0)
            last_pool = nc.gpsimd.memset(dum_t[:], 0)

            wx_t = constp.tile([C, W1C], BF16, tag="wx")
            w1_t = wx_t[:].rearrange("c (kh m) -> c kh m", kh=KH)
            wk2_t = constp.tile([M1, (1 + KW) * C], BF16, tag="wk2")
            w2f_t = wk2_t[:, 0:C]  # [112, 128]
            # w45z[kw]: [112, 128], rows 16kw..16kw+16 = w45.T, else 0
            w45z_t = [wk2_t[:, (1 + kw) * C : (2 + kw) * C] for kw in range(KW)]

            # input DMAs on the scalar (Act) HW queue: Act is idle until the
            # first evac (~3us after these kicks), and this keeps the sync
            # queue free for the 4 gathers + 2 output DMAs (6 = ring depth).
            # weights first: the queue round-robins packets of all queued
            # transfers across its 16 engines, so whatever is queued first
            # completes first -- and w1/x44 gate the first real matmul.
            wx_dma = nc.scalar.dma_start(wx_t[:], wx_d.ap())
            in_dmas.append(wx_dma)
            x_dmas = []
            for a, b in X_DMAS:
                d = nc.scalar.dma_start(
                    xrp_t[:, PH + a : PH + b, :], xr_d.ap()[:, a:b, :]
                )
                x_dmas.append(d)
                in_dmas.append(d)

            # stage-2 weights via SWDGE; tiny, needed ~6us in.  Its free wait
            # slot absorbs the p2s pad-memset dep so the dumps (which read
            # the pads) carry only their Act evac wait.
            wk2_dma = nc.gpsimd.dma_start(wk2_t[:], wk2_d.ap())
            in_dmas.append(wk2_dma)
            add_dep_helper(wk2_dma.ins, pad_ms.ins, sync=True, reason="import")

            # gathered+shifted stage-1 rows, still 74 wide: the gather is a
            # flat contiguous read per (kw,co) block offset by +3kw, so row
            # tails hold wrap garbage -- matmuls only read cols 0..56.
            p2a_t = p2ap.tile([M1, H - 8, WS], BF16)  # out rows 8..56
            outsb_t = outsp.tile([C, H * W], BF16)
            out_ap = out_d.ap()
            scr_ap = scr_d.ap()

            # dummy/absorb PSUM tile; writers ordered by PE program order
            dpt = psh.tile([C, 6, W], F32, tag="dum")

            def dummy_mm(rhs):
                return nc.tensor.matmul(
                    dpt[0:1, 0:1, 0:8].squeeze(1),
                    rhs[:, 0:1],
                    rhs[:, 0:8],
                    start=True,
                    stop=True,
                    skip_group_check=True,
                )

            # PE prewarm from the end of the NEFF preamble until input lands
            for _ in range(DUM_A):
                nc.tensor.matmul(
                    dpt[:, 0:6, :],
                    dum_t[:, 0:128],
                    dum_t[:, 0:336],
                    start=True,
                    stop=True,
                    skip_group_check=True,
                )

            def gather(di):
                (a, b), _ = DUMPS[di]
                n = b - a
                src = scr_ap.copy()
                v = src.ap
                v.clear()
                v.extend(
                    [
                        [CO * 16 * WS + DW, KW],
                        [16 * WS, CO],
                        [1, n * WS],
                    ]
                )
                src.offset = di * SLAB
                d = nc.sync.dma_start(p2a_t[:, a - 8 : b - 8, :], src)
                aux_dmas.append(d)
                return d

            def dump_ap(di, n):
                dst = scr_ap.copy()
                v = dst.ap
                v.clear()
                v.extend([[16 * WS, M1], [1, n * WS]])
                dst.offset = di * SLAB
                return dst

            # ---- stage 1 (reversed), with dumps+gathers trailing evacs ----
            evacs = []
            for ci, (a, b) in enumerate(S1_CHUNKS):
                pt = ps1.tile([M1, 8, W], F32, tag="p1")
                # kh order: opener must read only already-imported regions so
                # it carries just the PSUM-reuse wait (Matmult fits ONE wait).
                # Chunks 3 and 5 start right below an x-DMA boundary (padded
                # rows 32 / 16), so their kh=2 window would first-touch the
                # new region -- open with kh=4 (their topmost, oldest rows)
                # and let kh=0 carry the new DMA's wait on its free slot.
                if ci == 0:
                    kh_order = [0, 1, 2, 3, 4]
                elif ci == 3:
                    kh_order = [4, 0, 1, 2, 3]
                else:
                    kh_order = [2, 0, 1, 3, 4]
                for i, kh in enumerate(kh_order):
                    nc.tensor.matmul(
                        pt[:],
                        w1_t[:, kh, :],
                        xrp_t[:, a + DH * kh : b + DH * kh, :],
                        start=(i == 0),
                        stop=(i == KH - 1),
                        skip_group_check=True,
                    )
                ev = nc.scalar.copy(p2s_t[:, a:b, PW : PW + W], pt[:])
                evacs.append(ev)
                for di, ((dlo, dhi), after) in enumerate(DUMPS):
                    if after == ci:
                        dmp = nc.gpsimd.dma_start(
                            dump_ap(di, dhi - dlo),
                            p2s_t[:, dlo:dhi, :],
                        )
                        add_dep_helper(
                            dmp.ins,
                            (aux_dmas[-2] if di else wk2_dma).ins,
                            sync=False,
                            reason="order",
                        )
                        aux_dmas.append(dmp)
                        g = gather(di)
                        if di:
                            add_dep_helper(
                                g.ins, aux_dmas[-3].ins, sync=False, reason="order"
                            )

            # ---- stage 2 ----
            # Every PE group below is nosync-chained to the previous one and
            # every gpsimd DMA to the previous gpsimd DMA, so the scheduler
            # preserves the intended fire order.
            s2_mms = []
            s2_cps = []
            last_gp_dma = aux_dmas[-2]  # dump1 (gpsimd)

            def pin_pe(mm):
                if s2_mms:
                    add_dep_helper(
                        mm.ins, s2_mms[-1].ins, sync=False, reason="order"
                    )

            def emit_pair(di, absorb_gather):
                (a, b), _ = DUMPS[di]
                if absorb_gather:
                    ab = dummy_mm(p2a_t[:, a - 8 : a - 7, :].squeeze(1))
                    pin_pe(ab)
                    s2_mms.append(ab)
                for j in range(a, b, 8):
                    qt = ps2g.tile([C, 8, W], F32, tag="p2")
                    mm = nc.tensor.matmul(
                        qt[:],
                        w2f_t,
                        p2a_t[:, j - 8 : j, 0:W],
                        start=True,
                        stop=True,
                    )
                    pin_pe(mm)
                    s2_mms.append(mm)
                    s2_cps.append(
                        nc.vector.tensor_copy(
                            outsb_t[:, j * W : (j + 8) * W], qt[:]
                        )
                    )

            def emit_mux(mi):
                (a, b), _ = MUX_ROWS[mi]
                qt = psm.tile([C, 8, W], F32, tag="pm")
                for i, kw in enumerate([3, 0, 1, 2, 4, 5, 6]):
                    mm = nc.tensor.matmul(
                        qt[:],
                        w45z_t[kw],
                        p2s_t[:, a:b, DW * kw : DW * kw + W],
                        start=(i == 0),
                        stop=(i == KW - 1),
                        skip_group_check=True,
                    )
                    if i == 0:
                        pin_pe(mm)
                    s2_mms.append(mm)
                s2_cps.append(
                    nc.vector.tensor_copy(outsb_t[:, a * W : b * W], qt[:])
                )

            def emit_out_dma(a, b):
                nonlocal last_gp_dma
                d = nc.gpsimd.dma_start(
                    out_ap[:, a * W : b * W], outsb_t[:, a * W : b * W]
                )
                add_dep_helper(
                    d.ins, last_gp_dma.ins, sync=False, reason="order"
                )
                last_gp_dma = d
                out_dmas.append(d)

            emit_pair(0, absorb_gather=False)
            emit_out_dma(40, 56)
            emit_mux(0)
            emit_out_dma(16, 24)
            emit_mux(1)
            emit_out_dma(8, 16)
            emit_mux(2)
            emit_out_dma(0, 8)
            emit_pair(1, absorb_gather=True)
            emit_out_dma(24, 40)

            # final PE dummy reads the last output copy's region: it waits the
            # DVE copy and is nosync-pinned after every stage-2 op, so PE's
            # final tick transitively implies all compute.  The SP absorb nops
            # then cover it plus the DMAs, leaving the tail Drain <= 1 wait.
            chain = dummy_mm(outsb_t[:, 0:448])
            for m in s2_mms + s2_cps:
                add_dep_helper(chain.ins, m.ins, sync=False, reason="tail")
            for dep in (
                [chain]
                + in_dmas
                + aux_dmas
                + out_dmas
                + [evacs[-1], last_pool]
            ):
                nop = nc.sync.nop(nofuse=True)
                add_dep_helper(nop.ins, dep.ins, sync=True, reason="absorb tick")
    return nc


def _get_nc():
    global _NC
    if _NC is None:
        _NC = _build_nc()
    return _NC


def _prep_inputs(x, w3, w4, w5):
    w45 = (w5.astype(np.float64) @ w4.astype(np.float64)).astype(np.float32)
    # w1p[c, kh, kw*CO+co] = w3[co, c, kh, kw]
    w1p = np.transpose(w3, (1, 2, 3, 0)).reshape(C, W1C)
    # w2f[kw*CO+co, o] = w45[o, co]
    w2f = np.tile(w45.T, (KW, 1))
    wk2 = np.zeros((M1, (1 + KW) * C), np.float32)
    wk2[:, 0:C] = w2f
    for kw in range(KW):
        wk2[CO * kw : CO * (kw + 1), (1 + kw) * C : (2 + kw) * C] = w45.T
    wx = w1p.astype(ml_dtypes.bfloat16)
    wk2 = wk2.astype(ml_dtypes.bfloat16)
    xr = x.astype(ml_dtypes.bfloat16)
    return wx, xr, wk2


def kernel(x, w3, w4, w5, trace=False):
    x = np.asarray(x, np.float32)
    w3 = np.asarray(w3, np.float32)
    w4 = np.asarray(w4, np.float32)
    w5 = np.asarray(w5, np.float32)
    wx, xr, wk2 = _prep_inputs(x, w3, w4, w5)
    scr0 = np.zeros(len(DUMPS) * SLAB, ml_dtypes.bfloat16)
    in_maps = [
        {
            "wx": wx,
            "xr": np.ascontiguousarray(xr[n]),
            "wk2": wk2,
            "scr": scr0,
        }
        for n in range(N)
    ]
    global _NC
    res = None
    last_err = None
    for attempt in range(6):
        if _NC is None:
            _NC = _build_nc(attempt)
        try:
            res = run_bass_kernel_spmd(
                _NC, in_maps, core_ids=list(range(N)), trace=trace
            )
            break
        except Exception as e:  # compile-schedule flake: rebuild perturbed
            last_err = e
            _NC = None
    if res is None:
        raise last_err
    out = np.stack(
        [
            np.asarray(res.results[n]["out"])
            .astype(np.float32)
            .reshape(C, H, W)
            for n in range(N)
        ]
    )
    if trace:
        return out, res
    return out


# revision 47
# speedup vs baseline: 1.9516x; 1.0694x over previous
"""Trainium2 Bass kernel for dilated 5x7 conv (128->16ch) + 1x1 (16->16) + 1x1 (16->128).

Strategy (data-parallel, 1 image per core across 8 cores):
  reference: y = conv_dilated(x, w3, dil=(2,3), pad=(4,9)); y = w4@y; y = w5@y
  Host folds w45 = w5 @ w4  [128, 16].

  kh-first decomposition (vs the kw-first v1): stage 1 contracts (c, kh)
  producing (kw,co)=112 channels -- 112/128 PE column utilization and only
  5 matmuls per 8-row chunk (35 total, 15.7k PE-elems vs 22k for kw-first).
  The leftover stage-2 shift is then along the W (free) axis, which makes
  every output chunk depend on exactly ONE stage-1 chunk (no cross-chunk
  row coupling):

  Per core, image x [128, 56, 56] row-padded into SBUF xrp [128, 64, 56]
  (pad rows memset once).  Stage 1 (TensorE), reversed row order, 7 chunks
  of 8 rows: for kh in 0..4 one PSUM-accumulating matmul, lhsT =
  w1p[:, kh, :] [c=128, (kw,co)=112], rhs = xrp[:, a+2kh : b+2kh, :]
  -> P[(kw,co), r, w].  Evacuate PSUM->SBUF p2s (f32->bf16) on ScalarE.
  Stage 2 needs p2a[(kw,co), h, w] = p2[(kw,co), h, w + 3kw - 9] (w-shift
  only).  Rows 8..56: dump evac'd rows into a zero-padded DRAM slab
  [112, rows, 74] (cols 9..65), gather back with a diagonal AP (+3 per kw
  block, legal on the DRAM side); slab zero padding supplies the
  out-of-range zeros, so one uniform rectangular gather per pair and ONE
  K=112 matmul per 8 output rows (lhsT = w2f[(kw,co), o]).
  Rows 0..8 (dependent on the last stage-1 chunk) skip the roundtrip:
  7 col-subrange matmuls (kw=3 opens full-width, others accumulate into
  their valid column window) reading p2s directly.
  Evacuate stage 2 (VectorE, f32->bf16) and DMA out per 8/16 rows.
  PE p-state prewarm: a few dummy matmuls bridge the preamble-to-input-DMA
  gap so the clock ramp starts as early as possible.
  Single-wait discipline: every matmul carries at most one semaphore wait
  (PSUM-reuse or data); extra data deps ride Ldweights or are absorbed by
  tiny preceding PE matmuls reading a sliver of the same writer's region.
"""

import os
import sys

import numpy as np

for _p in ("/opt/trn_rl_repo", "/root/.axon_site/_ro/trn_rl_repo"):
    if os.path.isdir(_p) and _p not in sys.path:
        sys.path.insert(0, _p)

import ml_dtypes  # noqa: E402

import concourse.bass as bass  # noqa: E402
import concourse.tile as tile  # noqa: E402
from concourse.tile_rust import add_dep_helper  # noqa: E402
from concourse import mybir  # noqa: E402
from concourse.bass_utils import run_bass_kernel_spmd  # noqa: E402

N, C, H, W = 8, 128, 56, 56
CO = 16
KH, KW = 5, 7
DH, DW = 2, 3
PH, PW = 4, 9
RP = H + 2 * PH  # 64 padded rows in xrp
WS = W + 2 * PW  # 74 slab width
M1 = KW * CO  # 112 stage-1 output channels (kw, co)
W1C = KH * M1  # 560
SLAB = M1 * 8 * WS + 32  # scratch slab stride (incl. gather-overrun slop)
# stage-1 chunks in out-row coords, processed in listed (reversed) order
S1_CHUNKS = [(48, 56), (40, 48), (32, 40), (24, 32), (16, 24), (8, 16), (0, 8)]
# x DMAs in x-row coords, first-needed first (split so chunk 1 never stalls
# on the input stream and the PE p-state ramp is not paused by idle gaps)
X_DMAS = [(44, 56), (28, 44), (0, 28)]
# dump slabs: (out-row range, stage-1 chunk index whose evac completes it).
# One 8-row slab per early evac: each roundtrip starts as soon as its rows
# exist, so all gathers land while stage 1 / the mux groups still run.  The
# gather's co-dim stride equals its flat run (8*74), so the AP merges.
DUMPS = [((48, 56), 0), ((40, 48), 1), ((32, 40), 2), ((24, 32), 3)]
# rows 0..24 skip the DRAM roundtrip: per 8-row group, 7 accumulating
# matmuls read shifted windows [3kw, 3kw+56) straight from the 74-wide p2s
# (zero pad cols supply the out-of-range zeros).  Listed in emission order
# with the stage-1 chunk index whose evac gates them.
MUX_ROWS = [((16, 24), 4), ((8, 16), 5), ((0, 8), 6)]
BF16 = mybir.dt.bfloat16
F32 = mybir.dt.float32

DUM_A = 7  # prewarm dummies bridging preamble -> first input DMA

_NC = None


def _build_nc(attempt=0):
    nc = bass.Bass(
        "TRN2",
        target_bir_lowering=False,
        debug=False,
        enable_asserts=False,
        num_devices=N,
    )
    wx_d = nc.dram_tensor("wx", [C, W1C], BF16, kind="ExternalInput")
    xr_d = nc.dram_tensor("xr", [C, H, W], BF16, kind="ExternalInput")
    wk2_d = nc.dram_tensor("wk2", [M1, (1 + KW) * C], BF16, kind="ExternalInput")
    # flat scratch: len(DUMPS) slabs of [112, 16, 74], each with its own
    # slop for the gather's flat-read overrun (<= 3*(KW-1) elements) so the
    # overrun never aliases the next slab (would add a WAR wait to dumps)
    scr_d = nc.dram_tensor(
        "scr", [len(DUMPS) * SLAB], BF16, kind="ExternalInput"
    )
    out_d = nc.dram_tensor("out", [C, H * W], BF16, kind="ExternalOutput")

    with tile.TileContext(nc) as tc:
        for _ in range(attempt):
            nc.sync.nop(nofuse=True)
        with (
            tc.tile_pool(name="const", bufs=1) as constp,
            tc.tile_pool(name="xin", bufs=1) as xinp,
            tc.tile_pool(name="p2s", bufs=1) as p2sp,
            tc.tile_pool(name="p2a", bufs=1) as p2ap,
            tc.tile_pool(name="outs", bufs=1) as outsp,
            tc.tile_pool(name="dum", bufs=1) as dump_,
            tc.tile_pool(name="ps1", bufs=2, space="PSUM") as ps1,
            tc.tile_pool(name="ps2", bufs=2, space="PSUM") as ps2g,
            tc.tile_pool(name="psm", bufs=3, space="PSUM") as psm,
            tc.tile_pool(name="psh", bufs=1, space="PSUM") as psh,
        ):
            in_dmas = []
            aux_dmas = []
            out_dmas = []
            # dummy source for PE prewarm; memset LAST on gpsimd so the first
            # dummy's dep imports the whole gpsimd clock (incl. pad memsets).
            dum_t = dump_.tile([C, 448], BF16, tag="dum")

            xrp_t = xinp.tile([C, RP, W], BF16, tag="xrp")
            # stage-1 output, 74 wide with zero pad cols: evacs write cols
            # 9..65, so dumps are fully contiguous per partition (fast DMA)
            # and the mux reads shifted windows [3kw, 3kw+56) directly.
            p2s_t = p2sp.tile([M1, H, WS], BF16)
            # dum_t memset FIRST so the prewarm dummies start right after the
            # preamble barrier (earlier PE busy-start = earlier clock ramp).
            # The later pad memsets are imported into the PE/gpsimd clocks by
            # a pinned dep on a stage-1 matmul / the wk2 DMA.
            nc.gpsimd.memset(dum_t[:], 0)
            nc.gpsimd.memset(xrp_t[:, 0:PH, :], 0)
            nc.gpsimd.memset(xrp_t[:, PH + H : RP, :], 0)
            nc.gpsimd.memset(p2s_t[:, :, 0:PW], 0)
            pad_ms = nc.gpsimd.memset(p2s_t[:, :, PW + W : WS], 0)
            last_pool = pad_ms

            wx_t = constp.tile([C, W1C], BF16, tag="wx")
            w1_t = wx_t[:].rearrange("c (kh m) -> c kh m", kh=KH)
            wk2_t = constp.tile([M1, (1 + KW) * C], BF16, tag="wk2")
            w2f_t = wk2_t[:, 0:C]  # [112, 128]
            # w45z[kw]: [112, 128], rows 16kw..16kw+16 = w45.T, else 0
            w45z_t = [wk2_t[:, (1 + kw) * C : (2 + kw) * C] for kw in range(KW)]

            # input DMAs on the scalar (Act) HW queue: Act is idle until the
            # first evac (~3us after these kicks), and this keeps the sync
            # queue free for the 4 gathers + 2 output DMAs (6 = ring depth).
            # The queue round-robins packets of all queued transfers across
            # its 16 engines, so smaller/earlier transfers complete first.
            # Chunk 0 is gated by w1[kh=0,1] + x rows 44:56 -- issue those
            # first, the rest of the weights next, the bulk x last.
            w1a_dma = nc.scalar.dma_start(
                wx_t[:, 0 : 2 * M1], wx_d.ap()[:, 0 : 2 * M1]
            )
            in_dmas.append(w1a_dma)
            x_dmas = []
            for a, b in X_DMAS:
                d = nc.scalar.dma_start(
                    xrp_t[:, PH + a : PH + b, :], xr_d.ap()[:, a:b, :]
                )
                x_dmas.append(d)
                in_dmas.append(d)
                if a == 44:
                    w1b_dma = nc.scalar.dma_start(
                        wx_t[:, 2 * M1 : W1C], wx_d.ap()[:, 2 * M1 : W1C]
                    )
                    in_dmas.append(w1b_dma)

            # stage-2 weights via SWDGE; tiny, needed ~6us in.  Its free wait
            # slot absorbs the p2s pad-memset dep so the dumps (which read
            # the pads) carry only their Act evac wait.
            wk2_dma = nc.gpsimd.dma_start(wk2_t[:], wk2_d.ap())
            in_dmas.append(wk2_dma)
            add_dep_helper(wk2_dma.ins, pad_ms.ins, sync=True, reason="import")

            # gathered+shifted stage-1 rows, still 74 wide: the gather is a
            # flat contiguous read per (kw,co) block offset by +3kw, so row
            # tails hold wrap garbage -- matmuls only read cols 0..56.
            p2a_t = p2ap.tile([M1, H - 8, WS], BF16)  # out rows 8..56
            outsb_t = outsp.tile([C, H * W], BF16)
            out_ap = out_d.ap()
            scr_ap = scr_d.ap()

            # dummy/absorb PSUM tile; writers ordered by PE program order
            dpt = psh.tile([C, 6, W], F32, tag="dum")

            def dummy_mm(rhs):
                return nc.tensor.matmul(
                    dpt[0:1, 0:1, 0:8].squeeze(1),
                    rhs[:, 0:1],
                    rhs[:, 0:8],
                    start=True,
                    stop=True,
                    skip_group_check=True,
                )

            # PE prewarm from the end of the NEFF preamble until input lands
            for _ in range(DUM_A):
                nc.tensor.matmul(
                    dpt[:, 0:6, :],
                    dum_t[:, 0:128],
                    dum_t[:, 0:336],
                    start=True,
                    stop=True,
                    skip_group_check=True,
                )

            def gather(di):
                (a, b), _ = DUMPS[di]
                n = b - a
                src = scr_ap.copy()
                v = src.ap
                v.clear()
                v.extend(
                    [
                        [CO * 8 * WS + DW, KW],
                        [8 * WS, CO],
                        [1, n * WS],
                    ]
                )
                src.offset = di * SLAB
                # last gather rides the SWDGE queue right behind its dump:
                # the sync HW queue's descriptor ring is only ~3 deep, and a
                # ring wait on top of the dump wait would not fit.
                eng = nc.gpsimd if di == len(DUMPS) - 1 else nc.sync
                d = eng.dma_start(p2a_t[:, a - 8 : b - 8, :], src)
                aux_dmas.append(d)
                return d

            def dump_ap(di, n):
                dst = scr_ap.copy()
                v = dst.ap
                v.clear()
                v.extend([[8 * WS, M1], [1, n * WS]])
                dst.offset = di * SLAB
                return dst

            # ---- stage 1 (reversed), with dumps+gathers trailing evacs ----
            evacs = []
            for ci, (a, b) in enumerate(S1_CHUNKS):
                pt = ps1.tile([M1, 8, W], F32, tag="p1")
                # kh order: opener must read only already-imported regions so
                # it carries just the PSUM-reuse wait (Matmult fits ONE wait).
                # Chunks 3 and 5 start right below an x-DMA boundary (padded
                # rows 32 / 16), so their kh=2 window would first-touch the
                # new region -- open with kh=4 (their topmost, oldest rows)
                # and let kh=0 carry the new DMA's wait on its free slot.
                if ci == 0:
                    kh_order = [0, 1, 2, 3, 4]
                elif ci == 3:
                    kh_order = [4, 0, 1, 2, 3]
                else:
                    kh_order = [2, 0, 1, 3, 4]
                for i, kh in enumerate(kh_order):
                    mm = nc.tensor.matmul(
                        pt[:],
                        w1_t[:, kh, :],
                        xrp_t[:, a + DH * kh : b + DH * kh, :],
                        start=(i == 0),
                        stop=(i == KH - 1),
                        skip_group_check=True,
                    )
                    if ci == 1 and i == KH - 1:
                        # import the p2s pad-col memsets into the PE clock so
                        # the mux groups' pad reads are pre-subsumed (this mm
                        # has a free wait slot)
                        add_dep_helper(
                            mm.ins, pad_ms.ins, sync=True, reason="import"
                        )
                ev = nc.scalar.copy(p2s_t[:, a:b, PW : PW + W], pt[:])
                evacs.append(ev)
                for di, ((dlo, dhi), after) in enumerate(DUMPS):
                    if after == ci:
                        dmp = nc.gpsimd.dma_start(
                            dump_ap(di, dhi - dlo),
                            p2s_t[:, dlo:dhi, :],
                        )
                        add_dep_helper(
                            dmp.ins,
                            (aux_dmas[-2] if di else wk2_dma).ins,
                            sync=False,
                            reason="order",
                        )
                        aux_dmas.append(dmp)
                        g = gather(di)
                        if di == len(DUMPS) - 1:
                            add_dep_helper(
                                g.ins, dmp.ins, sync=False, reason="order"
                            )
                        elif di:
                            add_dep_helper(
                                g.ins, aux_dmas[-3].ins, sync=False, reason="order"
                            )

            # ---- stage 2 ----
            # Every PE group below is nosync-chained to the previous one and
            # every gpsimd DMA to the previous gpsimd DMA, so the scheduler
            # preserves the intended fire order.
            s2_mms = []
            s2_cps = []
            last_gp_dma = aux_dmas[-1]  # gather3 (gpsimd)

            def pin_pe(mm):
                if s2_mms:
                    add_dep_helper(
                        mm.ins, s2_mms[-1].ins, sync=False, reason="order"
                    )

            def emit_pair(di, absorb_gather):
                (a, b), _ = DUMPS[di]
                if absorb_gather:
                    ab = dummy_mm(p2a_t[:, a - 8 : a - 7, :].squeeze(1))
                    pin_pe(ab)
                    s2_mms.append(ab)
                for j in range(a, b, 8):
                    qt = ps2g.tile([C, 8, W], F32, tag="p2")
                    mm = nc.tensor.matmul(
                        qt[:],
                        w2f_t,
                        p2a_t[:, j - 8 : j, 0:W],
                        start=True,
                        stop=True,
                    )
                    pin_pe(mm)
                    s2_mms.append(mm)
                    s2_cps.append(
                        nc.vector.tensor_copy(
                            outsb_t[:, j * W : (j + 8) * W], qt[:]
                        )
                    )

            def emit_mux(mi):
                (a, b), _ = MUX_ROWS[mi]
                qt = psm.tile([C, 8, W], F32, tag="pm")
                for i, kw in enumerate([3, 0, 1, 2, 4, 5, 6]):
                    mm = nc.tensor.matmul(
                        qt[:],
                        w45z_t[kw],
                        p2s_t[:, a:b, DW * kw : DW * kw + W],
                        start=(i == 0),
                        stop=(i == KW - 1),
                        skip_group_check=True,
                    )
                    if i == 0:
                        pin_pe(mm)
                    s2_mms.append(mm)
                s2_cps.append(
                    nc.vector.tensor_copy(outsb_t[:, a * W : b * W], qt[:])
                )

            def emit_out_dma(a, b):
                nonlocal last_gp_dma
                d = nc.gpsimd.dma_start(
                    out_ap[:, a * W : b * W], outsb_t[:, a * W : b * W]
                )
                add_dep_helper(
                    d.ins, last_gp_dma.ins, sync=False, reason="order"
                )
                last_gp_dma = d
                out_dmas.append(d)

            emit_mux(0)
            emit_mux(1)
            emit_mux(2)
            emit_out_dma(0, 24)
            emit_pair(0, absorb_gather=False)
            emit_pair(1, absorb_gather=False)
            emit_pair(2, absorb_gather=True)
            emit_pair(3, absorb_gather=True)
            emit_out_dma(24, 56)

            # final PE dummy reads the last output copy's region: it waits the
            # DVE copy and is nosync-pinned after every stage-2 op, so PE's
            # final tick transitively implies all compute.  The SP absorb nops
            # then cover it plus the DMAs, leaving the tail Drain <= 1 wait.
            chain = dummy_mm(outsb_t[:, 0:448])
            for m in s2_mms + s2_cps:
                add_dep_helper(chain.ins, m.ins, sync=False, reason="tail")
            for dep in (
                [chain]
                + in_dmas
                + aux_dmas
                + out_dmas
                + [evacs[-1], last_pool]
            ):
                nop = nc.sync.nop(nofuse=True)
                add_dep_helper(nop.ins, dep.ins, sync=True, reason="absorb tick")
    return nc


def _get_nc():
    global _NC
    if _NC is None:
        _NC = _build_nc()
    return _NC


def _prep_inputs(x, w3, w4, w5):
    w45 = (w5.astype(np.float64) @ w4.astype(np.float64)).astype(np.float32)
    # w1p[c, kh, kw*CO+co] = w3[co, c, kh, kw]
    w1p = np.transpose(w3, (1, 2, 3, 0)).reshape(C, W1C)
    # w2f[kw*CO+co, o] = w45[o, co]
    w2f = np.tile(w45.T, (KW, 1))
    wk2 = np.zeros((M1, (1 + KW) * C), np.float32)
    wk2[:, 0:C] = w2f
    for kw in range(KW):
        wk2[CO * kw : CO * (kw + 1), (1 + kw) * C : (2 + kw) * C] = w45.T
    wx = w1p.astype(ml_dtypes.bfloat16)
    wk2 = wk2.astype(ml_dtypes.bfloat16)
    xr = x.astype(ml_dtypes.bfloat16)
    return wx, xr, wk2


def kernel(x, w3, w4, w5, trace=False):
    x = np.asarray(x, np.float32)
    w3 = np.asarray(w3, np.float32)
    w4 = np.asarray(w4, np.float32)
    w5 = np.asarray(w5, np.float32)
    wx, xr, wk2 = _prep_inputs(x, w3, w4, w5)
    scr0 = np.zeros(len(DUMPS) * SLAB, ml_dtypes.bfloat16)
    in_maps = [
        {
            "wx": wx,
            "xr": np.ascontiguousarray(xr[n]),
            "wk2": wk2,
            "scr": scr0,
        }
        for n in range(N)
    ]
    global _NC
    res = None
    last_err = None
    for attempt in range(6):
        if _NC is None:
            _NC = _build_nc(attempt)
        try:
            res = run_bass_kernel_spmd(
                _NC, in_maps, core_ids=list(range(N)), trace=trace
            )
            break
        except Exception as e:  # compile-schedule flake: rebuild perturbed
            last_err = e
            _NC = None
    if res is None:
        raise last_err
    out = np.stack(
        [
            np.asarray(res.results[n]["out"])
            .astype(np.float32)
            .reshape(C, H, W)
            for n in range(N)
        ]
    )
    if trace:
        return out, res
    return out
